# revision 21
# baseline (speedup 1.0000x reference)
"""Trainium2 Bass kernel for nn_Attention_mid (attention-LSTM decoder).

Data-parallel over batch across 8 NeuronCores: B=512 -> 64 per core.
All inputs are taken FULL; sharding/layout prep happens on host, the
device kernel runs per-core with no collectives, outputs are gathered
on host.

v2 vs v1: projT kept SBUF-resident (no per-step DRAM streaming), proj
free dims ordered (j, t, b) so the per-step q broadcast-add runs in DVE
2x mode (in1 has stride-0 on t but stride-1 innermost b), LSTM gates
reordered [i f o g] to merge the sigmoid-via-tanh activations.

Math (per core, BL=64 batch rows, T=128, D=H=512, S=26 steps):
  projT[h, (j,t,b)] = sum_d W_i2h[h,d] * H[b,t,d]        (once, -> SBUF bf16)
  per step s:
    qT[h,(j,b)] = sum_h' W_h2h[h,h'] * h_prev[b,h'] + b_h2h[h]
    th        = tanh(projT + qT bcast over t)            (DVE 2x add, ACT tanh)
    eT[t,b]   = sum_h W_score[h] * th[h,(j,t,b)]         (PE, th t-cols as lhsT)
    alphaT    = softmax over t (no max-subtraction; e is bounded)
    ctxT[d,b] = sum_t H[b,t,d] * alphaT[t,b]             (PE, H tiles as lhsT)
    gates     = [ctx; h] @ W^T + onehot-emb + biases     (PE, fp32 PSUM)
    LSTM elementwise (sigmoid via 0.5*tanh(x/2)+0.5)
    g[b,s,:]  = h_new @ W_gen^T + b_gen
"""

import sys

for p in ("/opt/trn_rl_repo", "/opt/trn_rl_repo/concourse"):
    if p not in sys.path:
        sys.path.insert(0, p)

import numpy as np
import ml_dtypes

BF16 = ml_dtypes.bfloat16

# Problem constants (hardcoded per contest contract)
B_FULL = 512
N_CORES = 8
T = 128
D = 512
H = 512
MID = 38
BOT = 38
S = 26  # batch_max_length + 1
P = 128  # SBUF partitions
HC = H // P  # 4 h-chunks
DC = D // P  # 4 d-chunks
KEMB = MID + BOT + 1  # 77: onehot-mid, onehot-bot, ones row (biases)


def build_kernel(nc, BL, steps=S):
    """Trace the per-core kernel into `nc` (a bacc.Bacc). Returns nothing.

    DRAM parameter names (all per-core shapes):
      hdbt   bf16 [128, DC*NB]   batch_H^T, d-chunk-major, free=(chunk, t*BL+b)
      hnat   bf16 [128, BL*D]    batch_H natural, part=t, free=(b, d)
      wi2hT  bf16 [128, DC*H]    W_i2h^T  [d-chunk part, (chunk, h)]
      wh2hT  bf16 [128, HC*H]    W_h2h^T  [h'-chunk part, (chunk, h)]
      bh2h   f32  [128, HC]      b_h2h chunks as columns
      wscore bf16 [128, HC*32]   W_score chunks at cols j*32
      wlstmT bf16 [128, 8*2048]  [ctx;h]-feature-chunk-major LSTM weights^T
                                 (gate order i,f,o,g)
      ohT    bf16 [KEMB, steps*BL]  per-step augmented onehot^T (mid/bot/ones)
      wembT  bf16 [KEMB, 2048]   [emb_mid^T; emb_bot^T; b_ih+b_hh] (i,f,o,g)
      wgenT  bf16 [128, HC*38]   W_gen^T chunks
      bgenB  f32  [BL, 38]       b_gen broadcast
      onesc  bf16 [128, 1]       ones column (softmax sum lhsT)
      onesr  f32  [1, 128]       ones row (softmax bcast lhsT)
      id64   bf16 [64, 64]       identity (PE transposes)
    Output:
      g      f32  [BL, steps*38]
    """
    import dataclasses

    import concourse.bass as bass
    import concourse.mybir as mybir
    import concourse.tile as tile
    from contextlib import ExitStack

    fp32 = mybir.dt.float32
    bf16 = mybir.dt.bfloat16
    AF = mybir.ActivationFunctionType
    ALU = mybir.AluOpType

    NB = BL * T
    SLAB_B = min(8, BL)  # batch rows per slab
    NSLAB = BL // SLAB_B

    hdbt_d = nc.declare_dram_parameter("hdbt", [P, DC * NB], bf16, isOutput=False)
    hnat_d = nc.declare_dram_parameter("hnat", [P, BL * D], bf16, isOutput=False)
    wi2hT_d = nc.declare_dram_parameter("wi2hT", [P, DC * H], bf16, isOutput=False)
    wh2hT_d = nc.declare_dram_parameter("wh2hT", [P, HC * H], bf16, isOutput=False)
    bh2h_d = nc.declare_dram_parameter("bh2h", [P, HC], fp32, isOutput=False)
    wscore_d = nc.declare_dram_parameter("wscore", [P, HC * 32], bf16, isOutput=False)
    wlstmT_d = nc.declare_dram_parameter("wlstmT", [P, 8 * 2048], bf16, isOutput=False)
    ohT_d = nc.declare_dram_parameter("ohT", [KEMB, steps * BL], bf16, isOutput=False)
    wembT_d = nc.declare_dram_parameter("wembT", [KEMB, 2048], bf16, isOutput=False)
    wgenT_d = nc.declare_dram_parameter("wgenT", [P, HC * 38], bf16, isOutput=False)
    bgenB_d = nc.declare_dram_parameter("bgenB", [BL, 38], fp32, isOutput=False)
    onesc_d = nc.declare_dram_parameter("onesc", [P, 1], bf16, isOutput=False)
    onesr_d = nc.declare_dram_parameter("onesr", [1, P], fp32, isOutput=False)
    id64_d = nc.declare_dram_parameter("id64", [64, 64], bf16, isOutput=False)
    g_d = nc.declare_dram_parameter("g", [BL, steps * 38], fp32, isOutput=True)

    with tile.TileContext(nc) as tc, ExitStack() as ctx:
        const = ctx.enter_context(tc.tile_pool(name="const", bufs=1))

        def load_const(name, dram, shape, dtype):
            t = const.tile(shape, dtype, tag=name)
            nc.sync.dma_start(t[:, :], dram[:, :])
            return t

        wh2hT = load_const("wh2hT", wh2hT_d, [P, HC * H], bf16)
        bh2h = load_const("bh2h", bh2h_d, [P, HC], fp32)
        wscore = load_const("wscore", wscore_d, [P, HC * 32], bf16)
        wlstmT = load_const("wlstmT", wlstmT_d, [P, 8 * 2048], bf16)
        ohT = load_const("ohT", ohT_d, [KEMB, steps * BL], bf16)
        wembT = load_const("wembT", wembT_d, [KEMB, 2048], bf16)
        wgenT = load_const("wgenT", wgenT_d, [P, HC * 38], bf16)
        bgenB = load_const("bgenB", bgenB_d, [BL, 38], fp32)
        onesc = load_const("onesc", onesc_d, [P, 1], bf16)
        onesr = load_const("onesr", onesr_d, [1, P], fp32)
        id64 = load_const("id64", id64_d, [64, 64], bf16)
        # Hsb loaded in 8 chunks so the DMA spreads across queues
        Hsb = const.tile([P, BL * D], bf16, tag="Hsb")
        CH = BL * D // 8
        for q in range(8):
            nc.sync.dma_start(
                Hsb[:, q * CH:(q + 1) * CH], hnat_d[:, q * CH:(q + 1) * CH]
            )

        # Resident projT: [128, (j, t, b)] bf16
        projT = const.tile([P, HC * NB], bf16, tag="projT")
        projT_4d = projT[:, :].rearrange("p (j t b) -> p j t b", j=HC, t=T)

        # Persistent state
        state = ctx.enter_context(tc.tile_pool(name="state", bufs=1))
        hT = state.tile([P, HC * BL], bf16, tag="hT")  # h^T chunks [h, b]
        c_sb = state.tile([BL, H], bf16, tag="c")
        qT = state.tile([P, HC * BL], bf16, tag="qT")  # [h, (j, b)] bf16
        expT = state.tile([P, BL], bf16, tag="expT")
        recip = state.tile([1, BL], fp32, tag="recip")
        alphaT = state.tile([P, BL], bf16, tag="alphaT")
        ctxT = state.tile([P, DC * BL], bf16, tag="ctxT")
        ifo_sb = state.tile([BL, 3 * H], bf16, tag="ifo")
        gg_sb = state.tile([BL, H], bf16, tag="gg")
        tcel = state.tile([BL, H], bf16, tag="tc")
        hnat = state.tile([BL, H], bf16, tag="hnat")

        nc.gpsimd.memset(hT[:, :], 0.0)
        nc.gpsimd.memset(c_sb[:, :], 0.0)

        # ---------------- Phase A: projT = W_i2h @ H^T (into SBUF) --------
        with (
            tc.tile_pool(name="prhs", bufs=10) as prhs,
            tc.tile_pool(name="pwA", bufs=1) as pwA,
            tc.tile_pool(name="psA", bufs=4, space="PSUM") as psA,
        ):
            wi2hT = pwA.tile([P, DC * H], bf16, tag="wi2hT")
            nc.sync.dma_start(wi2hT[:, :], wi2hT_d[:, :])
            for n in range(NB // 512):
                rt = []
                for k in range(DC):
                    r = prhs.tile([P, 512], bf16, tag="prhs")
                    nc.sync.dma_start(r[:, :], hdbt_d[:, k * NB + n * 512:][:, :512])
                    rt.append(r)
                for m in range(HC):
                    ps = psA.tile([P, 512], fp32, tag="psA")
                    for k in range(DC):
                        nc.tensor.matmul(
                            ps[:, :],
                            wi2hT[:, k * H + m * P: k * H + (m + 1) * P],
                            rt[k][:, :],
                            start=(k == 0),
                            stop=(k == DC - 1),
                        )
                    dst = projT[:, m * NB + n * 512:][:, :512]
                    if (n * HC + m) % 2 == 0:
                        nc.vector.tensor_copy(dst, ps[:, :])
                    else:
                        nc.scalar.copy(dst, ps[:, :])

        # ---------------- Phase B: the 26 recurrent steps ----------------
        thp = ctx.enter_context(tc.tile_pool(name="thp", bufs=3))
        gout = ctx.enter_context(tc.tile_pool(name="gout", bufs=2))
        ps_sm = ctx.enter_context(tc.tile_pool(name="ps_sm", bufs=1, space="PSUM"))
        ps_str = ctx.enter_context(tc.tile_pool(name="ps_str", bufs=2, space="PSUM"))
        ps_ctx = ctx.enter_context(tc.tile_pool(name="ps_ctx", bufs=1, space="PSUM"))
        ps_gt = ctx.enter_context(tc.tile_pool(name="ps_gt", bufs=1, space="PSUM"))

        for s in range(steps):
            # -- q^T = W_h2h @ h^T + b_h2h (per-partition bias in the copy) --
            qps = ps_sm.tile([P, HC * BL], fp32, tag="sm")
            for m in range(HC):
                for k in range(HC):
                    nc.tensor.matmul(
                        qps[:, m * BL:(m + 1) * BL],
                        wh2hT[:, k * H + m * P: k * H + (m + 1) * P],
                        hT[:, k * BL:(k + 1) * BL],
                        start=(k == 0),
                        stop=(k == HC - 1),
                    )
            for m in range(HC):
                nc.vector.tensor_scalar_add(
                    qT[:, m * BL:(m + 1) * BL],
                    qps[:, m * BL:(m + 1) * BL],
                    bh2h[:, m: m + 1],
                )

            # gates: h/embedding contributions first (gate order i,f,o,g)
            gps = ps_gt.tile([BL, 2048], fp32, tag="gt")
            for n in range(4):
                for k in range(HC):
                    nc.tensor.matmul(
                        gps[:, n * 512:(n + 1) * 512],
                        hT[:, k * BL:(k + 1) * BL],
                        wlstmT[:, (4 + k) * 2048 + n * 512: (4 + k) * 2048 + (n + 1) * 512],
                        start=(k == 0),
                        stop=False,
                    )
                nc.tensor.matmul(
                    gps[:, n * 512:(n + 1) * 512],
                    ohT[:, s * BL:(s + 1) * BL],
                    wembT[:, n * 512:(n + 1) * 512],
                    start=False,
                    stop=False,
                )

            ctxps = ps_ctx.tile([P, DC * BL], fp32, tag="ctx")
            th_tiles = {}

            def emit_add_tanh(sl):
                bs = slice(sl * SLAB_B, (sl + 1) * SLAB_B)
                th = thp.tile([P, HC * T * SLAB_B], bf16, tag="th")
                th_tiles[sl] = th
                th_v = th[:, :].rearrange("p (j t b) -> p j t b", j=HC, t=T)
                # broadcast-add q over t: in1 has stride-0 middle t dim and
                # stride-1 innermost b -> DVE 2x mode
                qv = qT[:, :].rearrange("p (j b) -> p j b", j=HC)[:, :, bs]
                q_bc = dataclasses.replace(
                    qv, ap=qv.ap[:-1] + [[0, T]] + qv.ap[-1:]
                )
                nc.vector.tensor_tensor(
                    out=th_v, in0=projT_4d[:, :, :, bs], in1=q_bc, op=ALU.add,
                )
                nc.scalar.activation(th[:, :], th[:, :], AF.Tanh)

            def emit_e_sm(sl):
                bs = slice(sl * SLAB_B, (sl + 1) * SLAB_B)
                th_v = th_tiles[sl][:, :].rearrange(
                    "p (j t b) -> p j t b", j=HC, t=T
                )
                eps = ps_str.tile([P, SLAB_B], fp32, tag="str")
                for bi in range(SLAB_B):
                    for j in range(HC):
                        nc.tensor.matmul(
                            eps[:, bi: bi + 1],
                            th_v[:, j, :, bi],
                            wscore[:, j * 32: j * 32 + 1],
                            start=(j == 0),
                            stop=(j == HC - 1),
                        )
                # softmax over t for this slab's columns (e bounded: no max)
                nc.scalar.activation(expT[:, bs], eps[:, :], AF.Exp)
                smps = ps_str.tile([P, 128], fp32, tag="str")
                nc.tensor.matmul(
                    smps[0:1, 0:SLAB_B], onesc[:, :], expT[:, bs],
                    start=True, stop=True,
                )
                nc.vector.reciprocal(recip[0:1, bs], smps[0:1, 0:SLAB_B])
                nc.tensor.matmul(
                    smps[:, 64:64 + SLAB_B], onesr[:, :], recip[0:1, bs],
                    start=True, stop=True,
                )
                nc.vector.tensor_tensor(
                    out=alphaT[:, bs], in0=expT[:, bs],
                    in1=smps[:, 64:64 + SLAB_B], op=ALU.mult,
                )

            def emit_ctx(sl):
                for bi in range(SLAB_B):
                    b = sl * SLAB_B + bi
                    for dj in range(DC):
                        nc.tensor.matmul(
                            ctxps[:, dj * BL + b: dj * BL + b + 1],
                            Hsb[:, b * D + dj * P: b * D + (dj + 1) * P],
                            alphaT[:, b: b + 1],
                            start=True,
                            stop=True,
                        )

            # software pipeline: emit add+tanh two slabs ahead of e/softmax and
            # three ahead of ctx, so no engine queue ever stalls mid-section
            for sl in range(NSLAB):
                emit_add_tanh(sl)
                if sl >= 2:
                    emit_e_sm(sl - 2)
                if sl >= 3:
                    emit_ctx(sl - 3)
            emit_e_sm(NSLAB - 2)
            emit_ctx(NSLAB - 3)
            emit_e_sm(NSLAB - 1)
            emit_ctx(NSLAB - 2)
            emit_ctx(NSLAB - 1)

            # ctx^T for the gate matmuls: per-chunk copy interleaved with the
            # gate passes so pass k starts while chunk k+1 still copies
            for k in range(DC):
                nc.vector.tensor_copy(
                    ctxT[:, k * BL:(k + 1) * BL], ctxps[:, k * BL:(k + 1) * BL]
                )
                for n in range(4):
                    nc.tensor.matmul(
                        gps[:, n * 512:(n + 1) * 512],
                        ctxT[:, k * BL:(k + 1) * BL],
                        wlstmT[:, k * 2048 + n * 512: k * 2048 + (n + 1) * 512],
                        start=False,
                        stop=(k == DC - 1),
                    )
            # sigmoid(x) = 0.5*tanh(x/2) + 0.5  (keeps exp_and_others table)
            # gate order i,f,o,g: one merged activation for i,f,o
            def warm_pe(dep_tile):
                # tiny matmul dependent on an LSTM intermediate: keeps the PE
                # HAM activity window non-idle through the serial tail so the
                # next step's matmuls run at full clock
                junk = ps_str.tile([P, 1], fp32, tag="str")
                nc.tensor.matmul(
                    junk[:, :], dep_tile[:, 0:P], onesc[0:BL, :],
                    start=True, stop=True,
                )

            nc.scalar.activation(ifo_sb[:, :], gps[:, 0:1536], AF.Tanh, scale=0.5)
            nc.vector.tensor_scalar(
                out=ifo_sb[:, :], in0=ifo_sb[:, :], scalar1=0.5, scalar2=0.5,
                op0=ALU.mult, op1=ALU.add,
            )
            warm_pe(ifo_sb)
            nc.scalar.activation(gg_sb[:, :], gps[:, 1536:2048], AF.Tanh)
            nc.vector.tensor_tensor(
                out=ifo_sb[:, H:2 * H], in0=ifo_sb[:, H:2 * H], in1=c_sb[:, :],
                op=ALU.mult,
            )  # f*c
            nc.vector.tensor_tensor(
                out=gg_sb[:, :], in0=ifo_sb[:, 0:H], in1=gg_sb[:, :], op=ALU.mult
            )  # i*g
            warm_pe(gg_sb)
            nc.vector.tensor_tensor(
                out=c_sb[:, :], in0=ifo_sb[:, H:2 * H], in1=gg_sb[:, :], op=ALU.add
            )
            nc.scalar.activation(tcel[:, :], c_sb[:, :], AF.Tanh)
            warm_pe(c_sb)
            nc.vector.tensor_tensor(
                out=hnat[:, :], in0=ifo_sb[:, 2 * H:], in1=tcel[:, :], op=ALU.mult
            )

            # -- h^T via PE transposes --
            hps = ps_sm.tile([P, HC * BL], bf16, tag="sm")
            for k in range(HC):
                nc.tensor.transpose(
                    hps[:, k * BL:(k + 1) * BL],
                    hnat[:, k * P:(k + 1) * P],
                    id64[0:BL, 0:BL],
                )
            nc.vector.tensor_copy(hT[:, :], hps[:, :])

            # -- per-step generator output --
            gop = ps_sm.tile([BL, 38], fp32, tag="sm")
            for k in range(HC):
                nc.tensor.matmul(
                    gop[:, :],
                    hT[:, k * BL:(k + 1) * BL],
                    wgenT[:, k * 38:(k + 1) * 38],
                    start=(k == 0),
                    stop=(k == HC - 1),
                )
            g_step = gout.tile([BL, 38], fp32, tag="g_step")
            nc.vector.tensor_tensor(
                out=g_step[:, :], in0=gop[:, :], in1=bgenB[:, :], op=ALU.add,
            )
            nc.sync.dma_start(g_d[:, s * 38:(s + 1) * 38], g_step[:, :])


def prep_core_inputs(bh, mid, bot, W_i2h, W_h2h, b_h2h, W_score,
                     W_ih, W_hh, b_ih, b_hh, W_gen, b_gen, BL, steps=S):
    """Host-side layout prep for one core's shard. bh: [BL, T, D] float32."""
    NB = BL * T
    f32 = np.float32

    def chunked_cols(mat, nchunk):
        # [nchunk*128, W] -> [128, nchunk*W] chunk-major in free dim
        w = mat.shape[1]
        return np.ascontiguousarray(
            mat.reshape(nchunk, P, w).transpose(1, 0, 2).reshape(P, nchunk * w)
        )

    # hdbt: [d, (t, b)] so phase A emits projT cols in (t, b) order
    hdbt = bh.transpose(2, 1, 0).reshape(D, NB)  # [d, (t,b)]
    hdbt = chunked_cols(hdbt, DC).astype(BF16)
    hnat = np.ascontiguousarray(
        bh.transpose(1, 0, 2).reshape(T, BL * D)
    ).astype(BF16)  # [t, (b,d)]

    wi2hT = chunked_cols(np.ascontiguousarray(W_i2h.T), DC).astype(BF16)
    wh2hT = chunked_cols(np.ascontiguousarray(W_h2h.T), HC).astype(BF16)
    bh2h_t = np.ascontiguousarray(b_h2h.reshape(HC, P).T).astype(f32)
    wscore_t = np.zeros((P, HC * 32), np.float32)
    wscore_t[:, 0::32] = W_score.reshape(HC, P).T
    wscore_t = wscore_t.astype(BF16)

    # LSTM weights with gate reorder [i f o g] (PyTorch order is i,f,g,o)
    perm = np.concatenate([
        np.arange(0, H),            # i
        np.arange(H, 2 * H),        # f
        np.arange(3 * H, 4 * H),    # o
        np.arange(2 * H, 3 * H),    # g
    ])
    W_ih_r = W_ih[perm]
    W_hh_r = W_hh[perm]
    b_r = (b_ih + b_hh)[perm]

    wlstm = np.concatenate([W_ih_r[:, :D], W_hh_r], axis=1).T  # [1024, 2048]
    wlstmT = chunked_cols(np.ascontiguousarray(wlstm), 8).astype(BF16)

    ohT = np.zeros((KEMB, steps * BL), f32)
    for s in range(steps):
        for b in range(BL):
            ohT[int(mid[b, s]), s * BL + b] = 1.0
            ohT[MID + int(bot[b, s]), s * BL + b] = 1.0
            ohT[KEMB - 1, s * BL + b] = 1.0
    ohT = ohT.astype(BF16)

    wembT = np.concatenate(
        [W_ih_r[:, D:D + MID].T, W_ih_r[:, D + MID:D + MID + BOT].T,
         b_r[None, :]], axis=0
    ).astype(BF16)  # [77, 2048]

    wgenT = chunked_cols(np.ascontiguousarray(W_gen.T), HC).astype(BF16)
    bgenB = np.tile(b_gen[None, :], (BL, 1)).astype(f32)

    return {
        "hdbt": hdbt,
        "hnat": hnat,
        "wi2hT": wi2hT,
        "wh2hT": wh2hT,
        "bh2h": bh2h_t,
        "wscore": wscore_t,
        "wlstmT": wlstmT,
        "ohT": ohT,
        "wembT": wembT,
        "wgenT": wgenT,
        "bgenB": bgenB,
        "onesc": np.ones((P, 1), BF16),
        "onesr": np.ones((1, P), f32),
        "id64": np.eye(64, dtype=BF16),
    }


def make_in_maps(batch_H, mid_prev_text, bot_text, W_i2h, W_h2h, b_h2h,
                 W_score, W_ih, W_hh, b_ih, b_hh, W_gen, b_gen,
                 batch_max_length, **_):
    assert int(batch_max_length) == S - 1
    batch_H = np.asarray(batch_H, np.float32)
    mid_prev_text = np.asarray(mid_prev_text)
    bot_text = np.asarray(bot_text)
    args = [np.asarray(a, np.float32) for a in
            (W_i2h, W_h2h, b_h2h, W_score, W_ih, W_hh, b_ih, b_hh, W_gen, b_gen)]
    BL = B_FULL // N_CORES
    in_maps = []
    for c in range(N_CORES):
        sl = slice(c * BL, (c + 1) * BL)
        in_maps.append(
            prep_core_inputs(
                batch_H[sl], mid_prev_text[sl], bot_text[sl], *args, BL=BL
            )
        )
    return in_maps


def run_on_cores(in_maps, **spmd_kwargs):
    from concourse import bacc
    from concourse.bass_utils import run_bass_kernel_spmd

    BL = B_FULL // N_CORES
    nc = bacc.Bacc("TRN2", target_bir_lowering=False, debug=False)
    build_kernel(nc, BL)
    if not nc.is_finalized():
        nc.finalize()
    res = run_bass_kernel_spmd(
        nc, in_maps, core_ids=list(range(N_CORES)), **spmd_kwargs
    )
    outs = [res.results[c]["g"].reshape(BL, S, MID) for c in range(N_CORES)]
    return np.concatenate(outs, axis=0).astype(np.float32), res


def kernel(**inputs):
    out, _ = run_on_cores(make_in_maps(**inputs))
    return out


# revision 22
# speedup vs baseline: 1.0533x; 1.0533x over previous
"""Trainium2 Bass kernel for nn_Attention_mid (attention-LSTM decoder).

Data-parallel over batch across 8 NeuronCores: B=512 -> 64 per core.
All inputs are taken FULL; sharding/layout prep happens on host, the
device kernel runs per-core with no collectives, outputs are gathered
on host.

v2 vs v1: projT kept SBUF-resident (no per-step DRAM streaming), proj
free dims ordered (j, t, b) so the per-step q broadcast-add runs in DVE
2x mode (in1 has stride-0 on t but stride-1 innermost b), LSTM gates
reordered [i f o g] to merge the sigmoid-via-tanh activations.

Math (per core, BL=64 batch rows, T=128, D=H=512, S=26 steps):
  projT[h, (j,t,b)] = sum_d W_i2h[h,d] * H[b,t,d]        (once, -> SBUF bf16)
  per step s:
    qT[h,(j,b)] = sum_h' W_h2h[h,h'] * h_prev[b,h'] + b_h2h[h]
    th        = tanh(projT + qT bcast over t)            (DVE 2x add, ACT tanh)
    eT[t,b]   = sum_h W_score[h] * th[h,(j,t,b)]         (PE, th t-cols as lhsT)
    alphaT    = softmax over t (no max-subtraction; e is bounded)
    ctxT[d,b] = sum_t H[b,t,d] * alphaT[t,b]             (PE, H tiles as lhsT)
    gates     = [ctx; h] @ W^T + onehot-emb + biases     (PE, fp32 PSUM)
    LSTM elementwise (sigmoid via 0.5*tanh(x/2)+0.5)
    g[b,s,:]  = h_new @ W_gen^T + b_gen
"""

import sys

for p in ("/opt/trn_rl_repo", "/opt/trn_rl_repo/concourse"):
    if p not in sys.path:
        sys.path.insert(0, p)

import numpy as np
import ml_dtypes

BF16 = ml_dtypes.bfloat16

# Problem constants (hardcoded per contest contract)
B_FULL = 512
N_CORES = 8
T = 128
D = 512
H = 512
MID = 38
BOT = 38
S = 26  # batch_max_length + 1
P = 128  # SBUF partitions
HC = H // P  # 4 h-chunks
DC = D // P  # 4 d-chunks
KEMB = MID + BOT + 1  # 77: onehot-mid, onehot-bot, ones row (biases)


def build_kernel(nc, BL, steps=S):
    """Trace the per-core kernel into `nc` (a bacc.Bacc). Returns nothing.

    DRAM parameter names (all per-core shapes):
      hdbt   bf16 [128, DC*NB]   batch_H^T, d-chunk-major, free=(chunk, t*BL+b)
      hnat   bf16 [128, BL*D]    batch_H natural, part=t, free=(b, d)
      wi2hT  bf16 [128, DC*H]    W_i2h^T  [d-chunk part, (chunk, h)]
      wh2hT  bf16 [128, HC*H]    W_h2h^T  [h'-chunk part, (chunk, h)]
      bh2h   f32  [128, HC]      b_h2h chunks as columns
      wscore bf16 [128, HC*32]   W_score chunks at cols j*32
      wlstmT bf16 [128, 8*2048]  [ctx;h]-feature-chunk-major LSTM weights^T
                                 (gate order i,f,o,g)
      ohT    bf16 [KEMB, steps*BL]  per-step augmented onehot^T (mid/bot/ones)
      wembT  bf16 [KEMB, 2048]   [emb_mid^T; emb_bot^T; b_ih+b_hh] (i,f,o,g)
      wgenT  bf16 [128, HC*38]   W_gen^T chunks
      bgenB  f32  [BL, 38]       b_gen broadcast
      onesc  bf16 [128, 1]       ones column (softmax sum lhsT)
      onesr  f32  [1, 128]       ones row (softmax bcast lhsT)
      id64   bf16 [64, 64]       identity (PE transposes)
    Output:
      g      f32  [BL, steps*38]
    """
    import dataclasses

    import concourse.bass as bass
    import concourse.mybir as mybir
    import concourse.tile as tile
    from contextlib import ExitStack

    fp32 = mybir.dt.float32
    bf16 = mybir.dt.bfloat16
    AF = mybir.ActivationFunctionType
    ALU = mybir.AluOpType

    NB = BL * T
    SLAB_B = min(8, BL)  # batch rows per slab
    NSLAB = BL // SLAB_B

    hdbt_d = nc.declare_dram_parameter("hdbt", [P, DC * NB], bf16, isOutput=False)
    hnat_d = nc.declare_dram_parameter("hnat", [P, BL * D], bf16, isOutput=False)
    wi2hT_d = nc.declare_dram_parameter("wi2hT", [P, DC * H], bf16, isOutput=False)
    wh2hT_d = nc.declare_dram_parameter("wh2hT", [P, HC * H], bf16, isOutput=False)
    bh2h_d = nc.declare_dram_parameter("bh2h", [P, HC], fp32, isOutput=False)
    wscore_d = nc.declare_dram_parameter("wscore", [P, HC * 32], bf16, isOutput=False)
    wlstmT_d = nc.declare_dram_parameter("wlstmT", [P, 8 * 2048], bf16, isOutput=False)
    ohT_d = nc.declare_dram_parameter("ohT", [KEMB, steps * BL], bf16, isOutput=False)
    wembT_d = nc.declare_dram_parameter("wembT", [KEMB, 2048], bf16, isOutput=False)
    wgenT_d = nc.declare_dram_parameter("wgenT", [P, HC * 38], bf16, isOutput=False)
    bgenB_d = nc.declare_dram_parameter("bgenB", [BL, 38], fp32, isOutput=False)
    onesc_d = nc.declare_dram_parameter("onesc", [P, 1], bf16, isOutput=False)
    onesr_d = nc.declare_dram_parameter("onesr", [1, P], fp32, isOutput=False)
    id64_d = nc.declare_dram_parameter("id64", [64, 64], bf16, isOutput=False)
    g_d = nc.declare_dram_parameter("g", [BL, steps * 38], fp32, isOutput=True)

    with tile.TileContext(nc) as tc, ExitStack() as ctx:
        const = ctx.enter_context(tc.tile_pool(name="const", bufs=1))

        def load_const(name, dram, shape, dtype):
            t = const.tile(shape, dtype, tag=name)
            nc.sync.dma_start(t[:, :], dram[:, :])
            return t

        wh2hT = load_const("wh2hT", wh2hT_d, [P, HC * H], bf16)
        bh2h = load_const("bh2h", bh2h_d, [P, HC], fp32)
        wscore = load_const("wscore", wscore_d, [P, HC * 32], bf16)
        wlstmT = load_const("wlstmT", wlstmT_d, [P, 8 * 2048], bf16)
        ohT = load_const("ohT", ohT_d, [KEMB, steps * BL], bf16)
        wembT = load_const("wembT", wembT_d, [KEMB, 2048], bf16)
        wgenT = load_const("wgenT", wgenT_d, [P, HC * 38], bf16)
        bgenB = load_const("bgenB", bgenB_d, [BL, 38], fp32)
        onesc = load_const("onesc", onesc_d, [P, 1], bf16)
        onesr = load_const("onesr", onesr_d, [1, P], fp32)
        id64 = load_const("id64", id64_d, [64, 64], bf16)
        # Hsb loaded in 8 chunks so the DMA spreads across queues
        Hsb = const.tile([P, BL * D], bf16, tag="Hsb")
        CH = BL * D // 8
        for q in range(8):
            nc.sync.dma_start(
                Hsb[:, q * CH:(q + 1) * CH], hnat_d[:, q * CH:(q + 1) * CH]
            )

        # Resident projT: [128, (j, t, b)] bf16
        projT = const.tile([P, HC * NB], bf16, tag="projT")
        projT_4d = projT[:, :].rearrange("p (j t b) -> p j t b", j=HC, t=T)

        # Persistent state
        state = ctx.enter_context(tc.tile_pool(name="state", bufs=1))
        hT = state.tile([P, HC * BL], bf16, tag="hT")  # h^T chunks [h, b]
        c_sb = state.tile([BL, H], bf16, tag="c")
        qT = state.tile([P, HC * BL], bf16, tag="qT")  # [h, (j, b)] bf16
        expT = state.tile([P, BL], bf16, tag="expT")
        recip = state.tile([1, BL], fp32, tag="recip")
        alphaT = state.tile([P, BL], bf16, tag="alphaT")
        ctxT = state.tile([P, DC * BL], bf16, tag="ctxT")
        ifo_sb = state.tile([BL, 3 * H], bf16, tag="ifo")
        gg_sb = state.tile([BL, H], bf16, tag="gg")
        tcel = state.tile([BL, H], bf16, tag="tc")
        hnat = state.tile([BL, H], bf16, tag="hnat")

        nc.gpsimd.memset(hT[:, :], 0.0)
        nc.gpsimd.memset(c_sb[:, :], 0.0)

        # ---------------- Phase A: projT = W_i2h @ H^T (into SBUF) --------
        with (
            tc.tile_pool(name="prhs", bufs=10) as prhs,
            tc.tile_pool(name="pwA", bufs=1) as pwA,
            tc.tile_pool(name="psA", bufs=4, space="PSUM") as psA,
        ):
            wi2hT = pwA.tile([P, DC * H], bf16, tag="wi2hT")
            nc.sync.dma_start(wi2hT[:, :], wi2hT_d[:, :])
            for n in range(NB // 512):
                rt = []
                for k in range(DC):
                    r = prhs.tile([P, 512], bf16, tag="prhs")
                    nc.sync.dma_start(r[:, :], hdbt_d[:, k * NB + n * 512:][:, :512])
                    rt.append(r)
                for m in range(HC):
                    ps = psA.tile([P, 512], fp32, tag="psA")
                    for k in range(DC):
                        nc.tensor.matmul(
                            ps[:, :],
                            wi2hT[:, k * H + m * P: k * H + (m + 1) * P],
                            rt[k][:, :],
                            start=(k == 0),
                            stop=(k == DC - 1),
                        )
                    dst = projT[:, m * NB + n * 512:][:, :512]
                    if (n * HC + m) % 2 == 0:
                        nc.vector.tensor_copy(dst, ps[:, :])
                    else:
                        nc.scalar.copy(dst, ps[:, :])

        # ---------------- Phase B: the 26 recurrent steps ----------------
        thp = ctx.enter_context(tc.tile_pool(name="thp", bufs=3))
        gout = ctx.enter_context(tc.tile_pool(name="gout", bufs=2))
        ps_sm = ctx.enter_context(tc.tile_pool(name="ps_sm", bufs=1, space="PSUM"))
        ps_str = ctx.enter_context(tc.tile_pool(name="ps_str", bufs=2, space="PSUM"))
        ps_ctx = ctx.enter_context(tc.tile_pool(name="ps_ctx", bufs=1, space="PSUM"))
        ps_gt = ctx.enter_context(tc.tile_pool(name="ps_gt", bufs=1, space="PSUM"))

        for s in range(steps):
            # -- q^T = W_h2h @ h^T + b_h2h (per-partition bias in the copy) --
            qps = ps_sm.tile([P, HC * BL], fp32, tag="sm")
            for m in range(HC):
                for k in range(HC):
                    nc.tensor.matmul(
                        qps[:, m * BL:(m + 1) * BL],
                        wh2hT[:, k * H + m * P: k * H + (m + 1) * P],
                        hT[:, k * BL:(k + 1) * BL],
                        start=(k == 0),
                        stop=(k == HC - 1),
                    )
            for m in range(HC):
                nc.vector.tensor_scalar_add(
                    qT[:, m * BL:(m + 1) * BL],
                    qps[:, m * BL:(m + 1) * BL],
                    bh2h[:, m: m + 1],
                )

            # gates: h/embedding contributions first (gate order i,f,o,g)
            gps = ps_gt.tile([BL, 2048], fp32, tag="gt")
            for n in range(4):
                for k in range(HC):
                    nc.tensor.matmul(
                        gps[:, n * 512:(n + 1) * 512],
                        hT[:, k * BL:(k + 1) * BL],
                        wlstmT[:, (4 + k) * 2048 + n * 512: (4 + k) * 2048 + (n + 1) * 512],
                        start=(k == 0),
                        stop=False,
                    )
                nc.tensor.matmul(
                    gps[:, n * 512:(n + 1) * 512],
                    ohT[:, s * BL:(s + 1) * BL],
                    wembT[:, n * 512:(n + 1) * 512],
                    start=False,
                    stop=False,
                )

            ctxps = ps_ctx.tile([P, DC * BL], fp32, tag="ctx")
            th_tiles = {}

            def emit_add_tanh(sl):
                bs = slice(sl * SLAB_B, (sl + 1) * SLAB_B)
                th = thp.tile([P, HC * T * SLAB_B], bf16, tag="th")
                th_tiles[sl] = th
                th_v = th[:, :].rearrange("p (j t b) -> p j t b", j=HC, t=T)
                # broadcast-add q over t: in1 has stride-0 middle t dim and
                # stride-1 innermost b -> DVE 2x mode
                qv = qT[:, :].rearrange("p (j b) -> p j b", j=HC)[:, :, bs]
                q_bc = dataclasses.replace(
                    qv, ap=qv.ap[:-1] + [[0, T]] + qv.ap[-1:]
                )
                nc.vector.tensor_tensor(
                    out=th_v, in0=projT_4d[:, :, :, bs], in1=q_bc, op=ALU.add,
                )
                nc.scalar.activation(th[:, :], th[:, :], AF.Tanh)

            def emit_e_sm(sl):
                bs = slice(sl * SLAB_B, (sl + 1) * SLAB_B)
                th_v = th_tiles[sl][:, :].rearrange(
                    "p (j t b) -> p j t b", j=HC, t=T
                )
                eps = ps_str.tile([P, SLAB_B], fp32, tag="str")
                for bi in range(SLAB_B):
                    for j in range(HC):
                        nc.tensor.matmul(
                            eps[:, bi: bi + 1],
                            th_v[:, j, :, bi],
                            wscore[:, j * 32: j * 32 + 1],
                            start=(j == 0),
                            stop=(j == HC - 1),
                        )
                # softmax over t for this slab's columns (e bounded: no max)
                nc.scalar.activation(expT[:, bs], eps[:, :], AF.Exp)
                smps = ps_str.tile([P, 128], fp32, tag="str")
                nc.tensor.matmul(
                    smps[0:1, 0:SLAB_B], onesc[:, :], expT[:, bs],
                    start=True, stop=True,
                )
                nc.vector.reciprocal(recip[0:1, bs], smps[0:1, 0:SLAB_B])
                nc.tensor.matmul(
                    smps[:, 64:64 + SLAB_B], onesr[:, :], recip[0:1, bs],
                    start=True, stop=True,
                )
                nc.vector.tensor_tensor(
                    out=alphaT[:, bs], in0=expT[:, bs],
                    in1=smps[:, 64:64 + SLAB_B], op=ALU.mult,
                )

            def emit_ctx(sl):
                for bi in range(SLAB_B):
                    b = sl * SLAB_B + bi
                    for dj in range(DC):
                        nc.tensor.matmul(
                            ctxps[:, dj * BL + b: dj * BL + b + 1],
                            Hsb[:, b * D + dj * P: b * D + (dj + 1) * P],
                            alphaT[:, b: b + 1],
                            start=True,
                            stop=True,
                        )

            # software pipeline: emit add+tanh two slabs ahead of e/softmax and
            # three ahead of ctx. Per iteration e_sm(sl-2) is emitted BEFORE
            # add_tanh(sl) so the DVE queue runs recip/mult for slab sl-2
            # ahead of the next 2.3us q-add (else ctx stalls through it).
            for sl in range(NSLAB):
                if sl >= 2:
                    emit_e_sm(sl - 2)
                emit_add_tanh(sl)
                if sl >= 3:
                    emit_ctx(sl - 3)
            emit_e_sm(NSLAB - 2)
            emit_ctx(NSLAB - 3)
            emit_e_sm(NSLAB - 1)
            emit_ctx(NSLAB - 2)
            emit_ctx(NSLAB - 1)

            # ctx^T for the gate matmuls: per-chunk copy interleaved with the
            # gate passes so pass k starts while chunk k+1 still copies
            for k in range(DC):
                nc.vector.tensor_copy(
                    ctxT[:, k * BL:(k + 1) * BL], ctxps[:, k * BL:(k + 1) * BL]
                )
                for n in range(4):
                    nc.tensor.matmul(
                        gps[:, n * 512:(n + 1) * 512],
                        ctxT[:, k * BL:(k + 1) * BL],
                        wlstmT[:, k * 2048 + n * 512: k * 2048 + (n + 1) * 512],
                        start=False,
                        stop=(k == DC - 1),
                    )
            # sigmoid(x) = 0.5*tanh(x/2) + 0.5  (keeps exp_and_others table)
            # gate order i,f,o,g: one merged activation for i,f,o
            def warm_pe(dep_tile):
                # tiny matmul dependent on an LSTM intermediate: keeps the PE
                # HAM activity window non-idle through the serial tail so the
                # next step's matmuls run at full clock
                junk = ps_str.tile([P, 1], fp32, tag="str")
                nc.tensor.matmul(
                    junk[:, :], dep_tile[:, 0:P], onesc[0:BL, :],
                    start=True, stop=True,
                )

            nc.scalar.activation(ifo_sb[:, :], gps[:, 0:1536], AF.Tanh, scale=0.5)
            nc.vector.tensor_scalar(
                out=ifo_sb[:, :], in0=ifo_sb[:, :], scalar1=0.5, scalar2=0.5,
                op0=ALU.mult, op1=ALU.add,
            )
            warm_pe(ifo_sb)
            nc.scalar.activation(gg_sb[:, :], gps[:, 1536:2048], AF.Tanh)
            nc.vector.tensor_tensor(
                out=ifo_sb[:, H:2 * H], in0=ifo_sb[:, H:2 * H], in1=c_sb[:, :],
                op=ALU.mult,
            )  # f*c
            nc.vector.tensor_tensor(
                out=gg_sb[:, :], in0=ifo_sb[:, 0:H], in1=gg_sb[:, :], op=ALU.mult
            )  # i*g
            warm_pe(gg_sb)
            nc.vector.tensor_tensor(
                out=c_sb[:, :], in0=ifo_sb[:, H:2 * H], in1=gg_sb[:, :], op=ALU.add
            )
            nc.scalar.activation(tcel[:, :], c_sb[:, :], AF.Tanh)
            warm_pe(c_sb)
            nc.vector.tensor_tensor(
                out=hnat[:, :], in0=ifo_sb[:, 2 * H:], in1=tcel[:, :], op=ALU.mult
            )

            # -- h^T via PE transposes --
            hps = ps_sm.tile([P, HC * BL], bf16, tag="sm")
            for k in range(HC):
                nc.tensor.transpose(
                    hps[:, k * BL:(k + 1) * BL],
                    hnat[:, k * P:(k + 1) * P],
                    id64[0:BL, 0:BL],
                )
            nc.vector.tensor_copy(hT[:, :], hps[:, :])

            # -- per-step generator output --
            gop = ps_sm.tile([BL, 38], fp32, tag="sm")
            for k in range(HC):
                nc.tensor.matmul(
                    gop[:, :],
                    hT[:, k * BL:(k + 1) * BL],
                    wgenT[:, k * 38:(k + 1) * 38],
                    start=(k == 0),
                    stop=(k == HC - 1),
                )
            g_step = gout.tile([BL, 38], fp32, tag="g_step")
            nc.vector.tensor_tensor(
                out=g_step[:, :], in0=gop[:, :], in1=bgenB[:, :], op=ALU.add,
            )
            nc.sync.dma_start(g_d[:, s * 38:(s + 1) * 38], g_step[:, :])


def prep_core_inputs(bh, mid, bot, W_i2h, W_h2h, b_h2h, W_score,
                     W_ih, W_hh, b_ih, b_hh, W_gen, b_gen, BL, steps=S):
    """Host-side layout prep for one core's shard. bh: [BL, T, D] float32."""
    NB = BL * T
    f32 = np.float32

    def chunked_cols(mat, nchunk):
        # [nchunk*128, W] -> [128, nchunk*W] chunk-major in free dim
        w = mat.shape[1]
        return np.ascontiguousarray(
            mat.reshape(nchunk, P, w).transpose(1, 0, 2).reshape(P, nchunk * w)
        )

    # hdbt: [d, (t, b)] so phase A emits projT cols in (t, b) order
    hdbt = bh.transpose(2, 1, 0).reshape(D, NB)  # [d, (t,b)]
    hdbt = chunked_cols(hdbt, DC).astype(BF16)
    hnat = np.ascontiguousarray(
        bh.transpose(1, 0, 2).reshape(T, BL * D)
    ).astype(BF16)  # [t, (b,d)]

    wi2hT = chunked_cols(np.ascontiguousarray(W_i2h.T), DC).astype(BF16)
    wh2hT = chunked_cols(np.ascontiguousarray(W_h2h.T), HC).astype(BF16)
    bh2h_t = np.ascontiguousarray(b_h2h.reshape(HC, P).T).astype(f32)
    wscore_t = np.zeros((P, HC * 32), np.float32)
    wscore_t[:, 0::32] = W_score.reshape(HC, P).T
    wscore_t = wscore_t.astype(BF16)

    # LSTM weights with gate reorder [i f o g] (PyTorch order is i,f,g,o)
    perm = np.concatenate([
        np.arange(0, H),            # i
        np.arange(H, 2 * H),        # f
        np.arange(3 * H, 4 * H),    # o
        np.arange(2 * H, 3 * H),    # g
    ])
    W_ih_r = W_ih[perm]
    W_hh_r = W_hh[perm]
    b_r = (b_ih + b_hh)[perm]

    wlstm = np.concatenate([W_ih_r[:, :D], W_hh_r], axis=1).T  # [1024, 2048]
    wlstmT = chunked_cols(np.ascontiguousarray(wlstm), 8).astype(BF16)

    ohT = np.zeros((KEMB, steps * BL), f32)
    for s in range(steps):
        for b in range(BL):
            ohT[int(mid[b, s]), s * BL + b] = 1.0
            ohT[MID + int(bot[b, s]), s * BL + b] = 1.0
            ohT[KEMB - 1, s * BL + b] = 1.0
    ohT = ohT.astype(BF16)

    wembT = np.concatenate(
        [W_ih_r[:, D:D + MID].T, W_ih_r[:, D + MID:D + MID + BOT].T,
         b_r[None, :]], axis=0
    ).astype(BF16)  # [77, 2048]

    wgenT = chunked_cols(np.ascontiguousarray(W_gen.T), HC).astype(BF16)
    bgenB = np.tile(b_gen[None, :], (BL, 1)).astype(f32)

    return {
        "hdbt": hdbt,
        "hnat": hnat,
        "wi2hT": wi2hT,
        "wh2hT": wh2hT,
        "bh2h": bh2h_t,
        "wscore": wscore_t,
        "wlstmT": wlstmT,
        "ohT": ohT,
        "wembT": wembT,
        "wgenT": wgenT,
        "bgenB": bgenB,
        "onesc": np.ones((P, 1), BF16),
        "onesr": np.ones((1, P), f32),
        "id64": np.eye(64, dtype=BF16),
    }


def make_in_maps(batch_H, mid_prev_text, bot_text, W_i2h, W_h2h, b_h2h,
                 W_score, W_ih, W_hh, b_ih, b_hh, W_gen, b_gen,
                 batch_max_length, **_):
    assert int(batch_max_length) == S - 1
    batch_H = np.asarray(batch_H, np.float32)
    mid_prev_text = np.asarray(mid_prev_text)
    bot_text = np.asarray(bot_text)
    args = [np.asarray(a, np.float32) for a in
            (W_i2h, W_h2h, b_h2h, W_score, W_ih, W_hh, b_ih, b_hh, W_gen, b_gen)]
    BL = B_FULL // N_CORES
    in_maps = []
    for c in range(N_CORES):
        sl = slice(c * BL, (c + 1) * BL)
        in_maps.append(
            prep_core_inputs(
                batch_H[sl], mid_prev_text[sl], bot_text[sl], *args, BL=BL
            )
        )
    return in_maps


def run_on_cores(in_maps, **spmd_kwargs):
    from concourse import bacc
    from concourse.bass_utils import run_bass_kernel_spmd

    BL = B_FULL // N_CORES
    nc = bacc.Bacc("TRN2", target_bir_lowering=False, debug=False)
    build_kernel(nc, BL)
    if not nc.is_finalized():
        nc.finalize()
    res = run_bass_kernel_spmd(
        nc, in_maps, core_ids=list(range(N_CORES)), **spmd_kwargs
    )
    outs = [res.results[c]["g"].reshape(BL, S, MID) for c in range(N_CORES)]
    return np.concatenate(outs, axis=0).astype(np.float32), res


def kernel(**inputs):
    out, _ = run_on_cores(make_in_maps(**inputs))
    return out
